# revision 15
# baseline (speedup 1.0000x reference)
"""Submanifold sparse 3D conv (160^3 grid, 400k voxels, 32->64ch, 3x3x3) on 8 trn2 cores.

Strategy: voxels split evenly across 8 cores (50k each), weights replicated.
The host builds the true rulebook: for each of the 27 kernel offsets it keeps
only the voxels whose neighbor at that offset exists (~9.8% for off-center
offsets), gathers their bf16 features, and packs them into 512-voxel tiles.
Three tiles stack per 96-partition block (bands at partition 0/32/64 - the
matmul AP limit), so input DMA ships ~12MB/core with zero wasted rows. The
device runs one standalone K=32 matmul per tile (psum bank per tile), drains
each PSUM tile to bf16 partials alternating Vector/Scalar engines, and
streams partials back in 36KB-per-partition chunks. The host scatter-adds
the 27 per-offset partials + bias into the final fp32 output (vectorized
fancy-index +=; each voxel appears at most once per offset). No GPSIMD, no
on-device gather. (fp8 partials tested: rel err 3e-2 > 2e-2 budget, so bf16.
Paired 2-bank drains tested: no drain saving, serialized matmuls - reverted.)
"""

import sys

for _p in ("/opt/trn_rl_repo",):
    if _p not in sys.path:
        sys.path.insert(0, _p)

import numpy as np

# ---- problem constants (hardcoded; kernel.py must be self-contained) ----
D = H = W = 160
N_VOX = 400_000
C_IN, C_OUT = 32, 64
CORES = 8
NPC = N_VOX // CORES  # 50_000 voxels per core

TILE = 512            # psum tile (one bank, fp32)
CB = 12               # blocks per DMA chunk
BANDS = 3             # tiles per block: matmul partition bases limited to 0/32/64

_OFFSETS = [(dz, dy, dx) for dz in (-1, 0, 1) for dy in (-1, 0, 1) for dx in (-1, 0, 1)]

_PROG_CACHE = {}
LAST_RESULTS = None
TRACE = False


def _build_program(tiles_per_offset):
    import concourse.bacc as bacc
    import concourse.tile as tile
    import concourse.mybir as mybir
    from contextlib import ExitStack

    dt = mybir.dt
    nc = bacc.Bacc("TRN2", target_bir_lowering=False, debug=False, num_devices=CORES)

    toffs = []
    for k, ntk in enumerate(tiles_per_offset):
        toffs.extend([k] * ntk)
    nblocks = -(-len(toffs) // BANDS)
    nblocks = -(-nblocks // CB) * CB  # pad to chunk multiple
    nt = nblocks * BANDS
    toffs.extend([0] * (nt - len(toffs)))  # dummy tiles (host ignores)
    nch = nblocks // CB

    gstream = nc.dram_tensor("gstream", [96, nblocks * TILE], dt.bfloat16, kind="ExternalInput").ap()
    wts = nc.dram_tensor("wts", [96, 27 * C_OUT], dt.bfloat16, kind="ExternalInput").ap()
    ostream = nc.dram_tensor("ostream", [C_OUT, nt * TILE], dt.bfloat16, kind="ExternalOutput").ap()

    with tile.TileContext(nc) as tc, ExitStack() as ctx:
        consts = ctx.enter_context(tc.tile_pool(name="consts", bufs=1))
        gp = ctx.enter_context(tc.tile_pool(name="gp", bufs=5))
        pp = ctx.enter_context(tc.tile_pool(name="psum", bufs=8, space="PSUM"))
        op = ctx.enter_context(tc.tile_pool(name="outp", bufs=3))

        wsb = consts.tile([96, 27 * C_OUT], dt.bfloat16)
        nc.sync.dma_start(wsb[:], wts[:])

        for ch in range(nch):
            j0 = ch * CB
            gt = gp.tile([96, CB * TILE], dt.bfloat16)
            nc.sync.dma_start(gt[:], gstream[:, j0 * TILE:(j0 + CB) * TILE])
            ot = op.tile([C_OUT, BANDS * CB * TILE], dt.bfloat16)
            for jj in range(CB):
                for b in range(BANDS):
                    i = (j0 + jj) * BANDS + b
                    k = toffs[i]
                    ps = pp.tile([C_OUT, TILE], dt.float32)
                    nc.tensor.matmul(
                        ps[:],
                        wsb[32 * b:32 * b + 32, k * C_OUT:(k + 1) * C_OUT],
                        gt[32 * b:32 * b + 32, jj * TILE:(jj + 1) * TILE],
                        start=True,
                        stop=True,
                    )
                    dst = ot[:, (jj * BANDS + b) * TILE:(jj * BANDS + b + 1) * TILE]
                    if i % 2 == 0:
                        nc.vector.tensor_copy(dst, ps[:])
                    else:
                        nc.scalar.copy(dst, ps[:])
            c0 = j0 * BANDS * TILE
            half = BANDS * (CB // 2) * TILE
            nc.sync.dma_start(ostream[:, c0:c0 + half], ot[:, 0:half])
            nc.sync.dma_start(ostream[:, c0 + half:c0 + 2 * half], ot[:, half:2 * half])

    nc.compile()
    return nc, nt


def _prep(features, coors, weight, bias):
    import ml_dtypes

    bf16 = ml_dtypes.bfloat16
    feats = np.asarray(features, np.float32).astype(bf16)  # [N, 32]
    co = np.asarray(coors, np.int32)
    wt = np.asarray(weight, np.float32)
    bi = np.asarray(bias, np.float32)
    n = feats.shape[0]
    assert n == N_VOX, n

    z = co[:, 1].astype(np.int64)
    y = co[:, 2].astype(np.int64)
    x = co[:, 3].astype(np.int64)

    grid = np.full(D * H * W, -1, np.int32)
    grid[(z * H + y) * W + x] = np.arange(n, dtype=np.int32)

    nbr = np.empty((27, n), np.int32)
    for k, (dz, dy, dx) in enumerate(_OFFSETS):
        nz, ny, nx = z + dz, y + dy, x + dx
        inb = (nz >= 0) & (nz < D) & (ny >= 0) & (ny < H) & (nx >= 0) & (nx < W)
        q = np.clip((nz * H + ny) * W + nx, 0, D * H * W - 1)
        nbr[k] = np.where(inb, grid[q], -1)

    # weights replicated on the three 32-partition bands
    wts_pk = np.empty((96, 27 * C_OUT), np.float32)
    for k in range(27):
        for b in range(BANDS):
            wts_pk[32 * b:32 * b + 32, C_OUT * k:C_OUT * (k + 1)] = wt[k]
    wts_pk = wts_pk.astype(bf16)

    # compacted column lists per (core, offset)
    cols_ck = [[None] * 27 for _ in range(CORES)]
    for c in range(CORES):
        nb = nbr[:, c * NPC:(c + 1) * NPC]
        for k in range(27):
            cols_ck[c][k] = np.nonzero(nb[k] >= 0)[0]
    tiles_per_offset = tuple(
        -(-max(len(cols_ck[c][k]) for c in range(CORES)) // TILE) for k in range(27)
    )
    cum = np.concatenate([[0], np.cumsum(tiles_per_offset)])

    key = tiles_per_offset
    if key not in _PROG_CACHE:
        _PROG_CACHE[key] = _build_program(tiles_per_offset)
    nc, nt = _PROG_CACHE[key]
    nblocks = nt // BANDS

    in_maps = []
    for c in range(CORES):
        nb = nbr[:, c * NPC:(c + 1) * NPC]
        gstream = np.zeros((96, nblocks * TILE), bf16)
        for k in range(27):
            cols = cols_ck[c][k]
            vals = feats[nb[k, cols]].T  # [32, L]
            L = len(cols)
            pos = 0
            for j2 in range(tiles_per_offset[k]):
                i = cum[k] + j2
                w = min(TILE, L - pos)
                if w <= 0:
                    break
                blk, b = divmod(i, BANDS)
                gstream[32 * b:32 * b + 32, blk * TILE:blk * TILE + w] = vals[:, pos:pos + w]
                pos += TILE
        in_maps.append({"gstream": gstream, "wts": wts_pk})
    return nc, in_maps, cols_ck, cum, bi


def _assemble(results, cols_ck, cum, bi):
    final = np.broadcast_to(bi, (N_VOX, C_OUT)).astype(np.float32).copy()
    for c in range(CORES):
        oc = np.asarray(results[c]["ostream"], dtype=np.float32)  # [64, nt*TILE]
        base = c * NPC
        for k in range(27):
            cols = cols_ck[c][k]
            s0 = cum[k] * TILE
            final[base + cols] += oc[:, s0:s0 + len(cols)].T
    return final


def kernel(features, coors, weight, bias, batch_size=1, **_kw):
    global LAST_RESULTS
    from concourse.bass_utils import run_bass_kernel_spmd

    nc, in_maps, cols_ck, cum, bi = _prep(features, coors, weight, bias)
    br = run_bass_kernel_spmd(nc, in_maps, list(range(CORES)), trace=TRACE)
    LAST_RESULTS = br
    return _assemble(br.results, cols_ck, cum, bi)
